# revision 18
# baseline (speedup 1.0000x reference)
# Deformable Conv2d (KS=3, stride=1, pad=1) on 8 NeuronCores, Bass/Tile.
#
# Sharding: data-parallel over batch. B=8, one batch element per core; each
# core holds the full (tiny) conv weights and its own (C,H,W) image. No
# halo / collectives needed.
#
# Per-core pipeline:
#   P0  cast-DMA x into a zero-padded bf16 SBUF image xpb with an extra
#       constant-ones channel (carries the conv bias).
#   P1  offset/modulator convs as 9 shift-matmuls (contraction over 65
#       channels = 64 image + ones*bias), PSUM -> off_sb[h, ch, w].
#   P2  per-tap channel contraction u_k[h*W+w, o] = sum_c x[c,h,w]*2*W[o,c,k]
#       (t-outer loop: one stationary x-column serves all 9 taps), stored
#       bf16 to a plain DRAM table utab[k, 1+h*W+w, o]; then DRAM->DRAM
#       strided copies build the vertical-pair table utabp[k, R(y,x), 128]
#       with row content [u(y,x,:) | u(y+1,x,:)], R(y,x) = (y+1)*130+(x+1)
#       covering y in [-1,127], x in [-1,128]; pads zeroed (NaN safety).
#   P3  offsets -> sampling positions: clip, floor (trunc trick), bilinear
#       corner weights * validity masks * sigmoid(mod), duplicated into bf16
#       pairs bp[h, k, j, w, 2] (j = jx*2+jy matches gathered element order);
#       int16 row indices idx = (y0c+1)*130 + x0c+1, y0c/x0c clamped to
#       [-1,127]; idx staged to DRAM and re-loaded in the gather's wrapped
#       (i%16, i//16) layout.
#   P4  for each w-chunk and tap k: ONE dma_gather fetches all 4 bilinear
#       corners (512B descriptor = pair-table rows r, r+1); DVE multiplies
#       by beta pairs; identity-matmul accumulates all 36 (k, corner-half)
#       terms into PSUM; slot sums on evacuation into ysb[h, w, o].
#   P5  one DMA writes ysb back as (o, h, w).

import numpy as np
import ml_dtypes
from contextlib import ExitStack

import concourse.bass as bass
import concourse.bacc as bacc
import concourse.tile as tile
import concourse.mybir as mybir
from concourse.bass_utils import run_bass_kernel_spmd
import bass_rust

FP32 = mybir.dt.float32
BF16 = mybir.dt.bfloat16
I16 = mybir.dt.int16

H = 128
C = 64
O = 64
K = 9
KS = 3
MAX_OFF = 32.0  # max(H, W)/4 with H=128
AL = mybir.AluOpType


def _ap(base, dims, offset):
    """Clone `base` AP with explicit [stride, count] dims and element offset."""
    c = base.copy()
    c.offset = offset
    c.ap = bass_rust.VecI64Pair([list(d) for d in dims])
    return c


def build_program(W=128, chunk_w=32, debug=False):
    HW = H * W
    ROWS = HW + 2          # plain per-tap table rows incl 1 pad row each side
    WP = W + 2             # pair-table columns: x in [-1, W]
    HP = H + 1             # pair-table rows: y in [-1, H-1]
    NRG = HP * WP          # pair-table rows per tap (16770)
    CW = chunk_w
    NI = CW * H            # gather indices per call
    assert W % CW == 0 and NI % 128 == 0

    nc = bacc.Bacc("TRN2", target_bir_lowering=False, debug=debug)

    xin = nc.dram_tensor("xin", [C, H, W], FP32, kind="ExternalInput")
    wconv = nc.dram_tensor("wconv", [C + 1, K, 27], BF16, kind="ExternalInput")
    wmat = nc.dram_tensor("wmat", [C, K, O], BF16, kind="ExternalInput")
    ident = nc.dram_tensor("ident", [128, 128], BF16, kind="ExternalInput")
    basey = nc.dram_tensor("basey", [H, K, W], FP32, kind="ExternalInput")
    basex = nc.dram_tensor("basex", [H, K, W], FP32, kind="ExternalInput")
    utab = nc.dram_tensor("utab", [K * ROWS * O + O], BF16, kind="Internal")
    utabp = nc.dram_tensor("utabp", [K * NRG * 128 + 128], BF16,
                           kind="Internal")
    idxd = nc.dram_tensor("idxd", [K * H * W], I16, kind="Internal")
    ydram = nc.dram_tensor("y", [O, H, W], FP32, kind="ExternalOutput")

    with tile.TileContext(nc) as tc, \
            tc.tile_pool(name="persist", bufs=1) as persist:

        wconv_sb = persist.tile([C + 1, K, 27], BF16, tag="wconv_sb")
        wmat_sb = persist.tile([C, K, O], BF16, tag="wmat_sb")
        ident_sb = persist.tile([128, 128], BF16, tag="ident_sb")
        basey_sb = persist.tile([H, K, W], FP32, tag="basey_sb")
        basex_sb = persist.tile([H, K, W], FP32, tag="basex_sb")
        off_sb = persist.tile([H, 27, W], FP32, tag="off_sb")
        ysb = persist.tile([H, O, W], FP32, tag="ysb")
        # beta pairs: [h, k, corner j = jx*2+jy, w, dup2] bf16
        bp = persist.tile([H, K, 4, W, 2], BF16, tag="bp")
        zpad = persist.tile([1, 65 * 128], BF16, tag="zpad")

        nc.sync.dma_start(out=wconv_sb[:], in_=wconv[:])
        nc.sync.dma_start(out=wmat_sb[:], in_=wmat[:])
        nc.sync.dma_start(out=ident_sb[:], in_=ident[:])
        nc.sync.dma_start(out=basey_sb[:], in_=basey[:])
        nc.sync.dma_start(out=basex_sb[:], in_=basex[:])

        nc.vector.memset(zpad[:], 0.0)
        # guard row past the last tap's table (touched by gather AP spans)
        nc.sync.dma_start(
            out=_ap(utabp[:], [[128, 1], [1, 128]], K * NRG * 128),
            in_=zpad[:, 0:128])
        for k in range(K):
            base = k * NRG * 128
            for col in (0, WP - 1):
                # x pad columns (full rows, both slots), y in [-1,127]
                nc.sync.dma_start(
                    out=_ap(utabp[:], [[WP * 128, 65], [1, 128]],
                            base + col * 128),
                    in_=zpad[:, 0:65 * 128])
                nc.sync.dma_start(
                    out=_ap(utabp[:], [[WP * 128, HP - 65], [1, 128]],
                            base + (65 * WP + col) * 128),
                    in_=zpad[:, 0:(HP - 65) * 128])
            # y = -1 row slot0, x in [0, W)
            nc.sync.dma_start(
                out=_ap(utabp[:], [[128, W], [1, 64]], base + 1 * 128),
                in_=zpad[:, 0:W * 64])
            # y = 127 row slot1 (u(128) pad), x in [0, W)
            nc.sync.dma_start(
                out=_ap(utabp[:], [[128, W], [1, 64]],
                        base + (H * WP + 1) * 128 + 64),
                in_=zpad[:, 0:W * 64])

        with ExitStack() as imgs:
            imgp = imgs.enter_context(tc.tile_pool(name="imgp", bufs=1))
            # P0: padded bf16 image + ones channel
            xpb = imgp.tile([C + 1, 130, W + 2], BF16, tag="xpb")
            nc.vector.memset(xpb[0:C, :, :], 0.0)
            nc.vector.memset(xpb[C:C + 1, :, :], 1.0)
            nc.gpsimd.dma_start(out=xpb[0:C, 1:129, 1:W + 1], in_=xin[:])

            # ---- P1: offset/modulator convs ----
            with tc.tile_pool(name="convp", bufs=2, space="PSUM") as convp:
                TB = 4  # w columns per psum tile
                for t0 in range(0, W, TB):
                    pc = convp.tile([128, TB, 27], FP32, tag="pc")
                    for j in range(TB):
                        t = t0 + j
                        for s in range(K):
                            dy, dx = s // KS, s % KS
                            lhsT = xpb[0:C + 1, dy:dy + 128, t + dx]
                            nc.tensor.matmul(
                                pc[:, j, :], lhsT, wconv_sb[:, s, :],
                                start=(s == 0), stop=(s == K - 1),
                            )
                    # psum (j, ch) -> off_sb (ch, w)
                    nc.vector.tensor_copy(
                        off_sb[:, :, t0:t0 + TB].rearrange("p c w -> p w c"),
                        pc[:],
                    )

            # ---- P2: u tables (t-outer, stationary x column reused) ----
            with (
                tc.tile_pool(name="usb", bufs=2) as upool,
                tc.tile_pool(name="up", bufs=2, space="PSUM") as upsum,
            ):
                UT = 3  # w columns per psum round; [128,4,512] = 4 banks
                for t0 in range(0, W, UT):
                    n = min(UT, W - t0)
                    pu = upsum.tile([128, 4, 512], FP32, tag="pu")
                    for i in range(n):
                        t = t0 + i
                        lhsT = xpb[0:C, 1:129, t + 1]
                        nc.tensor.matmul(
                            pu[:, i, :], lhsT, wmat_sb[:, 0:8, :],
                            start=True, stop=True,
                        )
                        nc.tensor.matmul(
                            pu[:, 3, i * 64:(i + 1) * 64], lhsT,
                            wmat_sb[:, 8, :],
                            start=True, stop=True,
                        )
                    u_sb = upool.tile([128, K, UT, 64], BF16, tag="u_sb")
                    nc.scalar.copy(
                        u_sb[:, 0:8, 0:n, :],
                        pu[:, 0:n, :].rearrange("p i (k o) -> p k i o", o=64),
                    )
                    nc.vector.tensor_copy(
                        u_sb[:, 8, 0:n, :],
                        pu[:, 3, 0:n * 64].rearrange("p (i o) -> p i o", o=64),
                    )
                    # one DMA per round: all taps, rows 1 + h*W + (t0..t0+n)
                    nc.sync.dma_start(
                        out=_ap(utab[:],
                                [[W * O, 128], [ROWS * O, K], [1, n * O]],
                                (0 * ROWS + 1) * O + t0 * O),
                        in_=u_sb[:, :, 0:n, :],
                    )

        # ---- P2b: build vertical-pair table via DRAM->DRAM copies ----
        for k in range(K):
            src0 = k * ROWS * O + O          # u(0, 0)
            dst = k * NRG * 128
            # slot0: rows R(y, x) <- u(y, x), y in [0,127]
            nc.sync.dma_start(
                out=_ap(utabp[:], [[WP * 128, H], [128, W], [1, 64]],
                        dst + (1 * WP + 1) * 128),
                in_=_ap(utab[:], [[W * O, H], [O, W], [1, O]], src0),
            )
            # slot1: rows R(y', x) <- u(y'+1, x), y' in [-1,126]
            nc.scalar.dma_start(
                out=_ap(utabp[:], [[WP * 128, H], [128, W], [1, 64]],
                        dst + (0 * WP + 1) * 128 + 64),
                in_=_ap(utab[:], [[W * O, H], [O, W], [1, O]], src0),
            )

        # ---- P3: beta weights + gather indices ----
        with tc.tile_pool(name="scr", bufs=1) as scr:
            def S(tag, dt=FP32):
                return scr.tile([H, K, W], dt, tag=tag, name=tag)

            msig = S("msig")
            py = S("py")
            px = S("px")
            tmp = S("tmp")
            fi = S("fi", I16)
            fyf = S("fyf")
            fxf = S("fxf")
            wy = S("wy")
            wx = S("wx")
            ga = S("ga")
            gb = S("gb")
            ay0 = S("ay0")
            ay1 = S("ay1")
            ax0 = S("ax0")
            ax1 = S("ax1")
            cc = S("cc")
            idxi = scr.tile([H, K, W], I16, tag="idxi")

            ts = nc.vector.tensor_scalar
            tt = nc.vector.tensor_tensor
            stt = nc.vector.scalar_tensor_tensor

            nc.scalar.activation(msig[:], off_sb[:, 18:27, :],
                                 mybir.ActivationFunctionType.Sigmoid)
            # clip offsets, add base grid
            offv = off_sb[:, 0:18, :].rearrange("p (a b) w -> p a b w", b=2)
            ts(py[:], offv[:, 0:9, 0, :], -MAX_OFF, MAX_OFF, AL.max, AL.min)
            ts(px[:], offv[:, 0:9, 1, :], -MAX_OFF, MAX_OFF, AL.max, AL.min)
            tt(py[:], py[:], basey_sb[:], AL.add)
            tt(px[:], px[:], basex_sb[:], AL.add)

            def floor_frac(p, ff, w_frac):
                # HW DVE float->int converts round-to-nearest:
                # rint(p+63.5)-64 == floor(p) for p >= -63 (integer-p ties
                # resolve to floor or floor-1, both bilinear-equivalent).
                ts(tmp[:], p[:], 63.5, None, AL.add)
                nc.vector.tensor_copy(fi[:], tmp[:])      # fp32 -> int16 trunc
                nc.vector.tensor_copy(ff[:], fi[:])       # back to fp32
                ts(ff[:], ff[:], -64.0, None, AL.add)
                tt(w_frac[:], p[:], ff[:], AL.subtract)

            floor_frac(py, fyf, wy)
            floor_frac(px, fxf, wx)

            def edge_weights(ff, hi0, a0, a1, w_frac):
                # a0 = (1-w)*[lo<=f<=hi], a1 = w*[lo-1<=f<=hi-1], lo=0
                ts(ga[:], ff[:], 0.0, None, AL.is_ge)
                ts(gb[:], ff[:], hi0, None, AL.is_le)
                tt(ga[:], ga[:], gb[:], AL.mult)                  # valid0
                ts(a0[:], w_frac[:], -1.0, 1.0, AL.mult, AL.add)  # 1-w
                tt(a0[:], a0[:], ga[:], AL.mult)
                ts(ga[:], ff[:], -1.0, None, AL.is_ge)
                ts(gb[:], ff[:], hi0 - 1.0, None, AL.is_le)
                tt(ga[:], ga[:], gb[:], AL.mult)                  # valid1
                tt(a1[:], w_frac[:], ga[:], AL.mult)

            edge_weights(fyf, 127.0, ay0, ay1, wy)
            edge_weights(fxf, float(W - 1), ax0, ax1, wx)

            tt(ay0[:], msig[:], ay0[:], AL.mult)   # m*(1-wy)*vy0
            tt(ay1[:], msig[:], ay1[:], AL.mult)

            # corner products -> bf16 pair-duplicated bp[h, k, jx*2+jy, w, 2]
            for jy, ayv in ((0, ay0), (1, ay1)):
                for jx, axv in ((0, ax0), (1, ax1)):
                    tt(cc[:], ayv[:], axv[:], AL.mult)
                    for d in range(2):
                        nc.vector.tensor_copy(
                            bp[:, :, jx * 2 + jy, :, d], cc[:])

            # index: (y0c+1)*WP + x0c + 1, clamps to [-1, 127]
            ts(ga[:], fyf[:], 127.0, -1.0, AL.min, AL.max)
            ts(gb[:], fxf[:], 127.0, -1.0, AL.min, AL.max)
            ts(gb[:], gb[:], float(WP + 1), None, AL.add)  # x0c + 131
            stt(wy[:], ga[:], float(WP), gb[:], AL.mult, AL.add)
            nc.vector.tensor_copy(idxi[:], wy[:])

            # stage indices to DRAM in (h, k, w) order
            nc.sync.dma_start(
                out=_ap(idxd[:], [[K * W, 128], [1, K * W]], 0),
                in_=idxi[:],
            )

        # ---- P4: gather + weighted combine ----
        gsem = nc.alloc_semaphore("gsem")
        with (
            tc.tile_pool(name="bpl", bufs=1) as bpool,
            tc.tile_pool(name="apl", bufs=2) as apool,
            tc.tile_pool(name="gp", bufs=2) as gp,
            tc.tile_pool(name="tp", bufs=2) as tp,
            tc.tile_pool(name="yp", bufs=1, space="PSUM") as yp,
        ):
            # Re-load indices into the gather's wrapped layout: index i lives
            # at [i%16, i//16] with i = w*128 + h, i.e. [h%16, w*8 + h//16].
            # DMA brings (r; j=h//16, w) with contiguous w; a strided DVE copy
            # interleaves to f = w*8 + j. Replicated into all 8 groups of 16
            # partitions for the 8 gpsimd cores.
            bplanes = {}
            for k in range(K):
                bt = bpool.tile([128, 8 * W], I16, tag=f"b{k}", name=f"b{k}")
                at = apool.tile([128, 8, W], I16, tag="at", name="at")
                for grp in range(8):
                    nc.sync.dma_start(
                        out=at[16 * grp:16 * (grp + 1), :, :],
                        in_=_ap(idxd[:],
                                [[K * W, 16], [16 * K * W, 8], [1, W]],
                                k * W),
                    )
                slb = bt[:]
                pstr = slb.ap[0][0]
                nc.vector.tensor_copy(
                    _ap(slb, [[pstr, 128], [1, 8], [8, W]], slb.offset),
                    at[:],
                )
                bplanes[k] = bt

            NJ = max(1, CW // 4)  # accumulation sub-blocks per chunk
            JW = CW // NJ         # w columns per sub-block
            gq = 0
            for c0 in range(0, W, CW):
                ypsums = [yp.tile([128, JW, 2, O], FP32, tag=f"yps{j}",
                                  name=f"yps{j}") for j in range(NJ)]
                n18 = 0
                for k in range(K):
                    g = gp.tile([128, CW, 256], BF16, tag="g", name="g")
                    # prepare descriptors only (engine retires at gen end,
                    # not DMA completion); trigger fires the transfer and the
                    # baked-in gsem counts data arrival for the consumer.
                    nc.gpsimd.dma_gather(
                        g[:],
                        _ap(utabp[:], [[128, NRG], [1, 256]],
                            k * NRG * 128),
                        bplanes[k][:, c0 * 8:(c0 + CW) * 8],
                        NI, NI,
                        256,
                        elem_step=128,
                        single_packet=False,
                        prepare_only=True,
                        sem=gsem,
                    )
                    nc.gpsimd.trigger_dma(count=None)
                    gq += 1
                    tv = tp.tile([128, CW, 4, 64], BF16, tag="tv", name="tv")
                    nc.vector.wait_ge(gsem, 16 * gq)
                    gv = g[:].rearrange("p c (j o) -> p c j o", j=4)
                    for j in range(4):
                        bview = (bp[:, k, j, c0:c0 + CW, :]
                                 .unsqueeze(2)
                                 .broadcast_to((H, CW, 32, 2)))
                        nc.vector.tensor_tensor(
                            tv[:, :, j, :].rearrange(
                                "p c (r d) -> p c r d", d=2),
                            gv[:, :, j, :].rearrange(
                                "p c (r d) -> p c r d", d=2),
                            bview, AL.mult,
                        )
                    for half in range(2):
                        for j in range(NJ):
                            nc.tensor.matmul(
                                ypsums[j][:],
                                ident_sb[:],
                                tv[:, j * JW:(j + 1) * JW,
                                   2 * half:2 * half + 2, :],
                                start=(n18 == 0), stop=(n18 == 17),
                            )
                        n18 += 1
                for j in range(NJ):
                    dst = ysb[:, :, c0 + j * JW: c0 + (j + 1) * JW]
                    nc.scalar.copy(
                        dst,
                        ypsums[j][:, :, 0, :].rearrange("p w o -> p o w"))
                    nc.vector.tensor_tensor(
                        dst,
                        ypsums[j][:, :, 1, :].rearrange("p w o -> p o w"),
                        dst, AL.add)

        # ---- P5: output ----
        nc.sync.dma_start(
            out=_ap(ydram[:], [[W, 128], [H * W, O], [1, W]], 0),
            in_=ysb[:],
        )

    nc.compile()
    return nc


def host_inputs(xb, offset_w, offset_b, mod_w, mod_b, weight, W=128):
    """Per-core input map for one batch element xb (C,H,W)."""
    wconv = np.zeros((C + 1, K, 27), np.float32)
    for s in range(K):
        dy, dx = s // KS, s % KS
        # out(h,t,j) = sum_c w[j,c,dy,dx] * xp[c, h+dy, t+dx]
        wconv[0:C, s, 0:18] = offset_w[:, :, dy, dx].T
        wconv[0:C, s, 18:27] = mod_w[:, :, dy, dx].T
    wconv[C, 0, 0:18] = offset_b
    wconv[C, 0, 18:27] = mod_b

    wmat = np.zeros((C, K, O), np.float32)
    for k in range(K):
        ky, kx = k // KS, k % KS
        wmat[:, k, :] = 2.0 * weight[:, :, ky, kx].T  # (c,o); x2 modulator fold

    hh = np.arange(H, dtype=np.float32)
    ww = np.arange(W, dtype=np.float32)
    ky = np.repeat(np.arange(KS), KS).astype(np.float32)
    kx = np.tile(np.arange(KS), KS).astype(np.float32)
    basey = np.broadcast_to(
        ky[None, :, None] + hh[:, None, None] - 1.0, (H, K, W))
    basex = np.broadcast_to(
        kx[None, :, None] + ww[None, None, :] - 1.0, (H, K, W))

    return {
        "xin": np.ascontiguousarray(xb, np.float32),
        "wconv": wconv.astype(ml_dtypes.bfloat16),
        "wmat": wmat.astype(ml_dtypes.bfloat16),
        "ident": np.eye(128, dtype=ml_dtypes.bfloat16),
        "basey": np.ascontiguousarray(basey, np.float32),
        "basex": np.ascontiguousarray(basex, np.float32),
    }


_prog_cache = {}


def _get_program(W=128, chunk_w=32):
    key = (W, chunk_w)
    if key not in _prog_cache:
        _prog_cache[key] = build_program(W=W, chunk_w=chunk_w)
    return _prog_cache[key]


def kernel(x, offset_w, offset_b, mod_w, mod_b, weight, trace=False):
    x = np.asarray(x, np.float32)
    B = x.shape[0]
    Wd = x.shape[3]
    nc = _get_program(W=Wd, chunk_w=min(32, Wd))
    in_maps = [
        host_inputs(x[b], np.asarray(offset_w, np.float32),
                    np.asarray(offset_b, np.float32),
                    np.asarray(mod_w, np.float32),
                    np.asarray(mod_b, np.float32),
                    np.asarray(weight, np.float32), W=Wd)
        for b in range(B)
    ]
    res = run_bass_kernel_spmd(nc, in_maps, list(range(B)), trace=trace)
    y = np.stack([res.results[b]["y"] for b in range(B)]).astype(np.float32)
    if trace:
        kernel.last_result = res
    return y


# revision 19
# speedup vs baseline: 1.0641x; 1.0641x over previous
# Deformable Conv2d (KS=3, stride=1, pad=1) on 8 NeuronCores, Bass/Tile.
#
# Sharding: data-parallel over batch. B=8, one batch element per core; each
# core holds the full (tiny) conv weights and its own (C,H,W) image. No
# halo / collectives needed.
#
# Per-core pipeline:
#   P0  cast-DMA x into a zero-padded bf16 SBUF image xpb with an extra
#       constant-ones channel (carries the conv bias).
#   P1  offset/modulator convs as 9 shift-matmuls (contraction over 65
#       channels = 64 image + ones*bias), PSUM -> off_sb[h, ch, w].
#   P2  per-tap channel contraction u_k[h*W+w, o] = sum_c x[c,h,w]*2*W[o,c,k]
#       (t-outer loop: one stationary x-column serves all 9 taps), stored
#       bf16 to a plain DRAM table utab[k, 1+h*W+w, o]; then DRAM->DRAM
#       strided copies build the vertical-pair table utabp[k, R(y,x), 128]
#       with row content [u(y,x,:) | u(y+1,x,:)], R(y,x) = (y+1)*130+(x+1)
#       covering y in [-1,127], x in [-1,128]; pads zeroed (NaN safety).
#   P3  offsets -> sampling positions: clip, floor (trunc trick), bilinear
#       corner weights * validity masks * sigmoid(mod), duplicated into bf16
#       pairs bp[h, k, j, w, 2] (j = jx*2+jy matches gathered element order);
#       int16 row indices idx = (y0c+1)*130 + x0c+1, y0c/x0c clamped to
#       [-1,127]; idx staged to DRAM and re-loaded in the gather's wrapped
#       (i%16, i//16) layout.
#   P4  for each w-chunk and tap k: ONE dma_gather fetches all 4 bilinear
#       corners (512B descriptor = pair-table rows r, r+1); DVE multiplies
#       by beta pairs; identity-matmul accumulates all 36 (k, corner-half)
#       terms into PSUM; slot sums on evacuation into ysb[h, w, o].
#   P5  one DMA writes ysb back as (o, h, w).

import numpy as np
import ml_dtypes
from contextlib import ExitStack

import concourse.bass as bass
import concourse.bacc as bacc
import concourse.tile as tile
import concourse.mybir as mybir
from concourse.bass_utils import run_bass_kernel_spmd
import bass_rust

FP32 = mybir.dt.float32
BF16 = mybir.dt.bfloat16
I16 = mybir.dt.int16

H = 128
C = 64
O = 64
K = 9
KS = 3
MAX_OFF = 32.0  # max(H, W)/4 with H=128
AL = mybir.AluOpType


def _ap(base, dims, offset):
    """Clone `base` AP with explicit [stride, count] dims and element offset."""
    c = base.copy()
    c.offset = offset
    c.ap = bass_rust.VecI64Pair([list(d) for d in dims])
    return c


def build_program(W=128, chunk_w=32, debug=False):
    HW = H * W
    ROWS = HW + 2          # plain per-tap table rows incl 1 pad row each side
    WP = W + 2             # pair-table columns: x in [-1, W]
    HP = H + 1             # pair-table rows: y in [-1, H-1]
    NRG = HP * WP          # pair-table rows per tap (16770)
    CW = chunk_w
    NI = CW * H            # gather indices per call
    assert W % CW == 0 and NI % 128 == 0

    nc = bacc.Bacc("TRN2", target_bir_lowering=False, debug=debug)

    xin = nc.dram_tensor("xin", [C, H, W], FP32, kind="ExternalInput")
    wconv = nc.dram_tensor("wconv", [C + 1, K, 27], BF16, kind="ExternalInput")
    wmat = nc.dram_tensor("wmat", [C, K, O], BF16, kind="ExternalInput")
    ident = nc.dram_tensor("ident", [128, 128], BF16, kind="ExternalInput")
    basey = nc.dram_tensor("basey", [H, K, W], FP32, kind="ExternalInput")
    basex = nc.dram_tensor("basex", [H, K, W], FP32, kind="ExternalInput")
    utab = nc.dram_tensor("utab", [K * ROWS * O + O], BF16, kind="Internal")
    utabp = nc.dram_tensor("utabp", [K * NRG * 128 + 128], BF16,
                           kind="Internal")
    idxd = nc.dram_tensor("idxd", [K * H * W], I16, kind="Internal")
    ydram = nc.dram_tensor("y", [O, H, W], FP32, kind="ExternalOutput")

    with tile.TileContext(nc) as tc, \
            tc.tile_pool(name="persist", bufs=1) as persist:

        wconv_sb = persist.tile([C + 1, K, 27], BF16, tag="wconv_sb")
        wmat_sb = persist.tile([C, K, O], BF16, tag="wmat_sb")
        ident_sb = persist.tile([128, 128], BF16, tag="ident_sb")
        basey_sb = persist.tile([H, K, W], FP32, tag="basey_sb")
        basex_sb = persist.tile([H, K, W], FP32, tag="basex_sb")
        off_sb = persist.tile([H, 27, W], FP32, tag="off_sb")
        ysb = persist.tile([H, O, W], FP32, tag="ysb")
        # beta pairs: [h, k, corner j = jx*2+jy, w, dup2] bf16
        bp = persist.tile([H, K, 4, W, 2], BF16, tag="bp")
        zpad = persist.tile([1, 65 * 128], BF16, tag="zpad")

        nc.sync.dma_start(out=wconv_sb[:], in_=wconv[:])
        nc.sync.dma_start(out=wmat_sb[:], in_=wmat[:])
        nc.sync.dma_start(out=ident_sb[:], in_=ident[:])
        nc.sync.dma_start(out=basey_sb[:], in_=basey[:])
        nc.sync.dma_start(out=basex_sb[:], in_=basex[:])

        nc.vector.memset(zpad[:], 0.0)
        # guard row past the last tap's table (touched by gather AP spans)
        nc.sync.dma_start(
            out=_ap(utabp[:], [[128, 1], [1, 128]], K * NRG * 128),
            in_=zpad[:, 0:128])
        for k in range(K):
            base = k * NRG * 128
            for col in (0, WP - 1):
                # x pad columns (full rows, both slots), y in [-1,127]
                nc.sync.dma_start(
                    out=_ap(utabp[:], [[WP * 128, 65], [1, 128]],
                            base + col * 128),
                    in_=zpad[:, 0:65 * 128])
                nc.sync.dma_start(
                    out=_ap(utabp[:], [[WP * 128, HP - 65], [1, 128]],
                            base + (65 * WP + col) * 128),
                    in_=zpad[:, 0:(HP - 65) * 128])
            # y = -1 row slot0, x in [0, W)
            nc.sync.dma_start(
                out=_ap(utabp[:], [[128, W], [1, 64]], base + 1 * 128),
                in_=zpad[:, 0:W * 64])
            # y = 127 row slot1 (u(128) pad), x in [0, W)
            nc.sync.dma_start(
                out=_ap(utabp[:], [[128, W], [1, 64]],
                        base + (H * WP + 1) * 128 + 64),
                in_=zpad[:, 0:W * 64])

        with ExitStack() as imgs:
            imgp = imgs.enter_context(tc.tile_pool(name="imgp", bufs=1))
            # P0: padded bf16 image + ones channel
            xpb = imgp.tile([C + 1, 130, W + 2], BF16, tag="xpb")
            nc.vector.memset(xpb[0:C, :, :], 0.0)
            nc.vector.memset(xpb[C:C + 1, :, :], 1.0)
            nc.gpsimd.dma_start(out=xpb[0:C, 1:129, 1:W + 1], in_=xin[:])

            # ---- P1: offset/modulator convs ----
            with tc.tile_pool(name="convp", bufs=2, space="PSUM") as convp:
                TB = 4  # w columns per psum tile
                for t0 in range(0, W, TB):
                    pc = convp.tile([128, TB, 27], FP32, tag="pc")
                    for j in range(TB):
                        t = t0 + j
                        for s in range(K):
                            dy, dx = s // KS, s % KS
                            lhsT = xpb[0:C + 1, dy:dy + 128, t + dx]
                            nc.tensor.matmul(
                                pc[:, j, :], lhsT, wconv_sb[:, s, :],
                                start=(s == 0), stop=(s == K - 1),
                            )
                    # psum (j, ch) -> off_sb (ch, w)
                    nc.vector.tensor_copy(
                        off_sb[:, :, t0:t0 + TB].rearrange("p c w -> p w c"),
                        pc[:],
                    )

            # ---- P2: u tables (t-outer, stationary x column reused) ----
            with (
                tc.tile_pool(name="usb", bufs=2) as upool,
                tc.tile_pool(name="up", bufs=2, space="PSUM") as upsum,
            ):
                UT = 3  # w columns per psum round; [128,4,512] = 4 banks
                for t0 in range(0, W, UT):
                    n = min(UT, W - t0)
                    pu = upsum.tile([128, 4, 512], FP32, tag="pu")
                    for i in range(n):
                        t = t0 + i
                        lhsT = xpb[0:C, 1:129, t + 1]
                        nc.tensor.matmul(
                            pu[:, i, :], lhsT, wmat_sb[:, 0:8, :],
                            start=True, stop=True,
                        )
                        nc.tensor.matmul(
                            pu[:, 3, i * 64:(i + 1) * 64], lhsT,
                            wmat_sb[:, 8, :],
                            start=True, stop=True,
                        )
                    u_sb = upool.tile([128, K, UT, 64], BF16, tag="u_sb")
                    nc.scalar.copy(
                        u_sb[:, 0:8, 0:n, :],
                        pu[:, 0:n, :].rearrange("p i (k o) -> p k i o", o=64),
                    )
                    nc.vector.tensor_copy(
                        u_sb[:, 8, 0:n, :],
                        pu[:, 3, 0:n * 64].rearrange("p (i o) -> p i o", o=64),
                    )
                    # one DMA per round: all taps, rows 1 + h*W + (t0..t0+n)
                    nc.sync.dma_start(
                        out=_ap(utab[:],
                                [[W * O, 128], [ROWS * O, K], [1, n * O]],
                                (0 * ROWS + 1) * O + t0 * O),
                        in_=u_sb[:, :, 0:n, :],
                    )

        # ---- P2b: build vertical-pair table via DRAM->DRAM copies ----
        for k in range(K):
            src0 = k * ROWS * O + O          # u(0, 0)
            dst = k * NRG * 128
            # slot0: rows R(y, x) <- u(y, x), y in [0,127]
            nc.sync.dma_start(
                out=_ap(utabp[:], [[WP * 128, H], [128, W], [1, 64]],
                        dst + (1 * WP + 1) * 128),
                in_=_ap(utab[:], [[W * O, H], [O, W], [1, O]], src0),
            )
            # slot1: rows R(y', x) <- u(y'+1, x), y' in [-1,126]
            nc.scalar.dma_start(
                out=_ap(utabp[:], [[WP * 128, H], [128, W], [1, 64]],
                        dst + (0 * WP + 1) * 128 + 64),
                in_=_ap(utab[:], [[W * O, H], [O, W], [1, O]], src0),
            )

        # ---- P3: beta weights + gather indices ----
        with tc.tile_pool(name="scr", bufs=1) as scr:
            def S(tag, dt=FP32):
                return scr.tile([H, K, W], dt, tag=tag, name=tag)

            msig = S("msig")
            py = S("py")
            px = S("px")
            tmp = S("tmp")
            fi = S("fi", I16)
            fyf = S("fyf")
            fxf = S("fxf")
            wy = S("wy")
            wx = S("wx")
            ga = S("ga")
            gb = S("gb")
            ay0 = S("ay0")
            ay1 = S("ay1")
            ax0 = S("ax0")
            ax1 = S("ax1")
            cc = S("cc")
            idxi = scr.tile([H, K, W], I16, tag="idxi")

            ts = nc.vector.tensor_scalar
            tt = nc.vector.tensor_tensor
            stt = nc.vector.scalar_tensor_tensor

            nc.scalar.activation(msig[:], off_sb[:, 18:27, :],
                                 mybir.ActivationFunctionType.Sigmoid)
            # clip offsets, add base grid
            offv = off_sb[:, 0:18, :].rearrange("p (a b) w -> p a b w", b=2)
            ts(py[:], offv[:, 0:9, 0, :], -MAX_OFF, MAX_OFF, AL.max, AL.min)
            ts(px[:], offv[:, 0:9, 1, :], -MAX_OFF, MAX_OFF, AL.max, AL.min)
            tt(py[:], py[:], basey_sb[:], AL.add)
            tt(px[:], px[:], basex_sb[:], AL.add)

            def floor_frac(p, ff, w_frac):
                # HW DVE float->int converts round-to-nearest:
                # rint(p+63.5)-64 == floor(p) for p >= -63 (integer-p ties
                # resolve to floor or floor-1, both bilinear-equivalent).
                ts(tmp[:], p[:], 63.5, None, AL.add)
                nc.vector.tensor_copy(fi[:], tmp[:])      # fp32 -> int16 trunc
                nc.vector.tensor_copy(ff[:], fi[:])       # back to fp32
                ts(ff[:], ff[:], -64.0, None, AL.add)
                tt(w_frac[:], p[:], ff[:], AL.subtract)

            floor_frac(py, fyf, wy)
            floor_frac(px, fxf, wx)

            def edge_weights(ff, hi0, a0, a1, w_frac):
                # a0 = (1-w)*[lo<=f<=hi], a1 = w*[lo-1<=f<=hi-1], lo=0
                ts(ga[:], ff[:], 0.0, None, AL.is_ge)
                ts(gb[:], ff[:], hi0, None, AL.is_le)
                tt(ga[:], ga[:], gb[:], AL.mult)                  # valid0
                ts(a0[:], w_frac[:], -1.0, 1.0, AL.mult, AL.add)  # 1-w
                tt(a0[:], a0[:], ga[:], AL.mult)
                ts(ga[:], ff[:], -1.0, None, AL.is_ge)
                ts(gb[:], ff[:], hi0 - 1.0, None, AL.is_le)
                tt(ga[:], ga[:], gb[:], AL.mult)                  # valid1
                tt(a1[:], w_frac[:], ga[:], AL.mult)

            edge_weights(fyf, 127.0, ay0, ay1, wy)
            edge_weights(fxf, float(W - 1), ax0, ax1, wx)

            tt(ay0[:], msig[:], ay0[:], AL.mult)   # m*(1-wy)*vy0
            tt(ay1[:], msig[:], ay1[:], AL.mult)

            # corner products -> bf16 pair-duplicated bp[h, k, jx*2+jy, w, 2]
            for jy, ayv in ((0, ay0), (1, ay1)):
                for jx, axv in ((0, ax0), (1, ax1)):
                    tt(cc[:], ayv[:], axv[:], AL.mult)
                    for d in range(2):
                        nc.vector.tensor_copy(
                            bp[:, :, jx * 2 + jy, :, d], cc[:])

            # index: (y0c+1)*WP + x0c + 1, clamps to [-1, 127]
            ts(ga[:], fyf[:], 127.0, -1.0, AL.min, AL.max)
            ts(gb[:], fxf[:], 127.0, -1.0, AL.min, AL.max)
            ts(gb[:], gb[:], float(WP + 1), None, AL.add)  # x0c + 131
            stt(wy[:], ga[:], float(WP), gb[:], AL.mult, AL.add)
            nc.vector.tensor_copy(idxi[:], wy[:])

            # stage indices to DRAM in (h, k, w) order
            nc.sync.dma_start(
                out=_ap(idxd[:], [[K * W, 128], [1, K * W]], 0),
                in_=idxi[:],
            )

        # ---- P4: gather + weighted combine ----
        with (
            tc.tile_pool(name="bpl", bufs=1) as bpool,
            tc.tile_pool(name="apl", bufs=2) as apool,
            tc.tile_pool(name="gp", bufs=2) as gp,
            tc.tile_pool(name="tp", bufs=2) as tp,
            tc.tile_pool(name="yp", bufs=1, space="PSUM") as yp,
        ):
            # Re-load indices into the gather's wrapped layout: index i lives
            # at [i%16, i//16] with i = w*128 + h, i.e. [h%16, w*8 + h//16].
            # DMA brings (r; j=h//16, w) with contiguous w; a strided DVE copy
            # interleaves to f = w*8 + j. Replicated into all 8 groups of 16
            # partitions for the 8 gpsimd cores.
            bplanes = {}
            for k in range(K):
                bt = bpool.tile([128, 8 * W], I16, tag=f"b{k}", name=f"b{k}")
                at = apool.tile([128, 8, W], I16, tag="at", name="at")
                for grp in range(8):
                    nc.sync.dma_start(
                        out=at[16 * grp:16 * (grp + 1), :, :],
                        in_=_ap(idxd[:],
                                [[K * W, 16], [16 * K * W, 8], [1, W]],
                                k * W),
                    )
                slb = bt[:]
                pstr = slb.ap[0][0]
                nc.vector.tensor_copy(
                    _ap(slb, [[pstr, 128], [1, 8], [8, W]], slb.offset),
                    at[:],
                )
                bplanes[k] = bt

            NJ = max(1, CW // 4)  # accumulation sub-blocks per chunk
            JW = CW // NJ         # w columns per sub-block
            gq = 0
            for c0 in range(0, W, CW):
                ypsums = [yp.tile([128, JW, 2, O], FP32, tag=f"yps{j}",
                                  name=f"yps{j}") for j in range(NJ)]
                n18 = 0
                for k in range(K):
                    g = gp.tile([128, CW, 256], BF16, tag="g", name="g")
                    nc.gpsimd.dma_gather(
                        g[:],
                        _ap(utabp[:], [[128, NRG], [1, 256]],
                            k * NRG * 128),
                        bplanes[k][:, c0 * 8:(c0 + CW) * 8],
                        NI, NI,
                        256,
                        elem_step=128,
                        single_packet=False,
                    )
                    gq += 1
                    tv = tp.tile([128, CW, 4, 64], BF16, tag="tv", name="tv")
                    gv = g[:].rearrange("p c (j o) -> p c j o", j=4)
                    for j in range(4):
                        bview = (bp[:, k, j, c0:c0 + CW, :]
                                 .unsqueeze(2)
                                 .broadcast_to((H, CW, 32, 2)))
                        nc.vector.tensor_tensor(
                            tv[:, :, j, :].rearrange(
                                "p c (r d) -> p c r d", d=2),
                            gv[:, :, j, :].rearrange(
                                "p c (r d) -> p c r d", d=2),
                            bview, AL.mult,
                        )
                    for half in range(2):
                        for j in range(NJ):
                            nc.tensor.matmul(
                                ypsums[j][:],
                                ident_sb[:],
                                tv[:, j * JW:(j + 1) * JW,
                                   2 * half:2 * half + 2, :],
                                start=(n18 == 0), stop=(n18 == 17),
                            )
                        n18 += 1
                for j in range(NJ):
                    dst = ysb[:, :, c0 + j * JW: c0 + (j + 1) * JW]
                    nc.scalar.copy(
                        dst,
                        ypsums[j][:, :, 0, :].rearrange("p w o -> p o w"))
                    nc.vector.tensor_tensor(
                        dst,
                        ypsums[j][:, :, 1, :].rearrange("p w o -> p o w"),
                        dst, AL.add)

        # ---- P5: output ----
        nc.sync.dma_start(
            out=_ap(ydram[:], [[W, 128], [H * W, O], [1, W]], 0),
            in_=ysb[:],
        )

    nc.compile()
    return nc


def host_inputs(xb, offset_w, offset_b, mod_w, mod_b, weight, W=128):
    """Per-core input map for one batch element xb (C,H,W)."""
    wconv = np.zeros((C + 1, K, 27), np.float32)
    for s in range(K):
        dy, dx = s // KS, s % KS
        # out(h,t,j) = sum_c w[j,c,dy,dx] * xp[c, h+dy, t+dx]
        wconv[0:C, s, 0:18] = offset_w[:, :, dy, dx].T
        wconv[0:C, s, 18:27] = mod_w[:, :, dy, dx].T
    wconv[C, 0, 0:18] = offset_b
    wconv[C, 0, 18:27] = mod_b

    wmat = np.zeros((C, K, O), np.float32)
    for k in range(K):
        ky, kx = k // KS, k % KS
        wmat[:, k, :] = 2.0 * weight[:, :, ky, kx].T  # (c,o); x2 modulator fold

    hh = np.arange(H, dtype=np.float32)
    ww = np.arange(W, dtype=np.float32)
    ky = np.repeat(np.arange(KS), KS).astype(np.float32)
    kx = np.tile(np.arange(KS), KS).astype(np.float32)
    basey = np.broadcast_to(
        ky[None, :, None] + hh[:, None, None] - 1.0, (H, K, W))
    basex = np.broadcast_to(
        kx[None, :, None] + ww[None, None, :] - 1.0, (H, K, W))

    return {
        "xin": np.ascontiguousarray(xb, np.float32),
        "wconv": wconv.astype(ml_dtypes.bfloat16),
        "wmat": wmat.astype(ml_dtypes.bfloat16),
        "ident": np.eye(128, dtype=ml_dtypes.bfloat16),
        "basey": np.ascontiguousarray(basey, np.float32),
        "basex": np.ascontiguousarray(basex, np.float32),
    }


_prog_cache = {}


def _get_program(W=128, chunk_w=32):
    key = (W, chunk_w)
    if key not in _prog_cache:
        _prog_cache[key] = build_program(W=W, chunk_w=chunk_w)
    return _prog_cache[key]


def kernel(x, offset_w, offset_b, mod_w, mod_b, weight, trace=False):
    x = np.asarray(x, np.float32)
    B = x.shape[0]
    Wd = x.shape[3]
    nc = _get_program(W=Wd, chunk_w=min(32, Wd))
    in_maps = [
        host_inputs(x[b], np.asarray(offset_w, np.float32),
                    np.asarray(offset_b, np.float32),
                    np.asarray(mod_w, np.float32),
                    np.asarray(mod_b, np.float32),
                    np.asarray(weight, np.float32), W=Wd)
        for b in range(B)
    ]
    res = run_bass_kernel_spmd(nc, in_maps, list(range(B)), trace=trace)
    y = np.stack([res.results[b]["y"] for b in range(B)]).astype(np.float32)
    if trace:
        kernel.last_result = res
    return y


# revision 20
# speedup vs baseline: 1.0815x; 1.0163x over previous
# Deformable Conv2d (KS=3, stride=1, pad=1) on 8 NeuronCores, Bass/Tile.
#
# Sharding: data-parallel over batch. B=8, one batch element per core; each
# core holds the full (tiny) conv weights and its own (C,H,W) image. No
# halo / collectives needed.
#
# Per-core pipeline:
#   P0  cast-DMA x into a zero-padded bf16 SBUF image xpb with an extra
#       constant-ones channel (carries the conv bias).
#   P1  offset/modulator convs as 9 shift-matmuls (contraction over 65
#       channels = 64 image + ones*bias), PSUM -> off_sb[h, ch, w].
#   P2  per-tap channel contraction u_k[h*W+w, o] = sum_c x[c,h,w]*2*W[o,c,k]
#       (t-outer loop: one stationary x-column serves all 9 taps), stored
#       bf16 to a plain DRAM table utab[k, 1+h*W+w, o]; then DRAM->DRAM
#       strided copies build the vertical-pair table utabp[k, R(y,x), 128]
#       with row content [u(y,x,:) | u(y+1,x,:)], R(y,x) = (y+1)*130+(x+1)
#       covering y in [-1,127], x in [-1,128]; pads zeroed (NaN safety).
#   P3  offsets -> sampling positions: clip, floor (trunc trick), bilinear
#       corner weights * validity masks * sigmoid(mod), duplicated into bf16
#       pairs bp[h, k, j, w, 2] (j = jx*2+jy matches gathered element order);
#       int16 row indices idx = (y0c+1)*130 + x0c+1, y0c/x0c clamped to
#       [-1,127]; idx staged to DRAM and re-loaded in the gather's wrapped
#       (i%16, i//16) layout.
#   P4  for each w-chunk and tap k: ONE dma_gather fetches all 4 bilinear
#       corners (512B descriptor = pair-table rows r, r+1); DVE multiplies
#       by beta pairs; identity-matmul accumulates all 36 (k, corner-half)
#       terms into PSUM; slot sums on evacuation into ysb[h, w, o].
#   P5  one DMA writes ysb back as (o, h, w).

import numpy as np
import ml_dtypes
from contextlib import ExitStack

import concourse.bass as bass
import concourse.bacc as bacc
import concourse.tile as tile
import concourse.mybir as mybir
from concourse.bass_utils import run_bass_kernel_spmd
import bass_rust

FP32 = mybir.dt.float32
BF16 = mybir.dt.bfloat16
I16 = mybir.dt.int16

H = 128
C = 64
O = 64
K = 9
KS = 3
MAX_OFF = 32.0  # max(H, W)/4 with H=128
AL = mybir.AluOpType


def _ap(base, dims, offset):
    """Clone `base` AP with explicit [stride, count] dims and element offset."""
    c = base.copy()
    c.offset = offset
    c.ap = bass_rust.VecI64Pair([list(d) for d in dims])
    return c


def build_program(W=128, chunk_w=32, debug=False):
    HW = H * W
    ROWS = HW + 2          # plain per-tap table rows incl 1 pad row each side
    WP = W + 2             # pair-table columns: x in [-1, W]
    HP = H + 1             # pair-table rows: y in [-1, H-1]
    NRG = HP * WP          # pair-table rows per tap (16770)
    CW = chunk_w
    NI = CW * H            # gather indices per call
    assert W % CW == 0 and NI % 128 == 0

    nc = bacc.Bacc("TRN2", target_bir_lowering=False, debug=debug)

    xin = nc.dram_tensor("xin", [C, H, W], FP32, kind="ExternalInput")
    wconv = nc.dram_tensor("wconv", [C + 1, K, 27], BF16, kind="ExternalInput")
    wmat = nc.dram_tensor("wmat", [C, K, O], BF16, kind="ExternalInput")
    ident = nc.dram_tensor("ident", [128, 128], BF16, kind="ExternalInput")
    basey = nc.dram_tensor("basey", [H, K, W], FP32, kind="ExternalInput")
    basex = nc.dram_tensor("basex", [H, K, W], FP32, kind="ExternalInput")
    utab = nc.dram_tensor("utab", [K * ROWS * O + O], BF16, kind="Internal")
    utabp = nc.dram_tensor("utabp", [K * NRG * 128 + 128], BF16,
                           kind="Internal")
    idxd = nc.dram_tensor("idxd", [K * H * W], I16, kind="Internal")
    ydram = nc.dram_tensor("y", [O, H, W], FP32, kind="ExternalOutput")

    with tile.TileContext(nc) as tc, \
            tc.tile_pool(name="persist", bufs=1) as persist:

        wconv_sb = persist.tile([C + 1, K, 27], BF16, tag="wconv_sb")
        wmat_sb = persist.tile([C, K, O], BF16, tag="wmat_sb")
        ident_sb = persist.tile([128, 128], BF16, tag="ident_sb")
        basey_sb = persist.tile([H, K, W], FP32, tag="basey_sb")
        basex_sb = persist.tile([H, K, W], FP32, tag="basex_sb")
        off_sb = persist.tile([H, 27, W], FP32, tag="off_sb")
        ysb = persist.tile([H, O, W], FP32, tag="ysb")
        # beta pairs: [h, k, corner j = jx*2+jy, w, dup2] bf16
        bp = persist.tile([H, K, 4, W, 2], BF16, tag="bp")
        zpad = persist.tile([1, 65 * 128], BF16, tag="zpad")

        nc.sync.dma_start(out=wconv_sb[:], in_=wconv[:])
        nc.sync.dma_start(out=wmat_sb[:], in_=wmat[:])
        nc.sync.dma_start(out=ident_sb[:], in_=ident[:])
        nc.sync.dma_start(out=basey_sb[:], in_=basey[:])
        nc.sync.dma_start(out=basex_sb[:], in_=basex[:])

        nc.vector.memset(zpad[:], 0.0)
        # guard row past the last tap's table (touched by gather AP spans)
        nc.sync.dma_start(
            out=_ap(utabp[:], [[128, 1], [1, 128]], K * NRG * 128),
            in_=zpad[:, 0:128])
        for k in range(K):
            base = k * NRG * 128
            for col in (0, WP - 1):
                # x pad columns (full rows, both slots), y in [-1,127]
                nc.sync.dma_start(
                    out=_ap(utabp[:], [[WP * 128, 65], [1, 128]],
                            base + col * 128),
                    in_=zpad[:, 0:65 * 128])
                nc.sync.dma_start(
                    out=_ap(utabp[:], [[WP * 128, HP - 65], [1, 128]],
                            base + (65 * WP + col) * 128),
                    in_=zpad[:, 0:(HP - 65) * 128])
            # y = -1 row slot0, x in [0, W)
            nc.sync.dma_start(
                out=_ap(utabp[:], [[128, W], [1, 64]], base + 1 * 128),
                in_=zpad[:, 0:W * 64])
            # y = 127 row slot1 (u(128) pad), x in [0, W)
            nc.sync.dma_start(
                out=_ap(utabp[:], [[128, W], [1, 64]],
                        base + (H * WP + 1) * 128 + 64),
                in_=zpad[:, 0:W * 64])

        with ExitStack() as imgs:
            imgp = imgs.enter_context(tc.tile_pool(name="imgp", bufs=1))
            # P0: padded bf16 image + ones channel
            xpb = imgp.tile([C + 1, 130, W + 2], BF16, tag="xpb")
            nc.vector.memset(xpb[0:C, :, :], 0.0)
            nc.vector.memset(xpb[C:C + 1, :, :], 1.0)
            nc.gpsimd.dma_start(out=xpb[0:C, 1:129, 1:W + 1], in_=xin[:])

            # ---- P1: offset/modulator convs ----
            with tc.tile_pool(name="convp", bufs=2, space="PSUM") as convp:
                TB = 4  # w columns per psum tile
                for t0 in range(0, W, TB):
                    pc = convp.tile([128, TB, 27], FP32, tag="pc")
                    for j in range(TB):
                        t = t0 + j
                        for s in range(K):
                            dy, dx = s // KS, s % KS
                            lhsT = xpb[0:C + 1, dy:dy + 128, t + dx]
                            nc.tensor.matmul(
                                pc[:, j, :], lhsT, wconv_sb[:, s, :],
                                start=(s == 0), stop=(s == K - 1),
                            )
                    # psum (j, ch) -> off_sb (ch, w)
                    nc.vector.tensor_copy(
                        off_sb[:, :, t0:t0 + TB].rearrange("p c w -> p w c"),
                        pc[:],
                    )

            # ---- P2: u tables (t-outer, stationary x column reused) ----
            with (
                tc.tile_pool(name="usb", bufs=2) as upool,
                tc.tile_pool(name="up", bufs=2, space="PSUM") as upsum,
            ):
                UT = 3  # w columns per psum round; [128,4,512] = 4 banks
                for t0 in range(0, W, UT):
                    n = min(UT, W - t0)
                    pu = upsum.tile([128, 4, 512], FP32, tag="pu")
                    for i in range(n):
                        t = t0 + i
                        lhsT = xpb[0:C, 1:129, t + 1]
                        nc.tensor.matmul(
                            pu[:, i, :], lhsT, wmat_sb[:, 0:8, :],
                            start=True, stop=True,
                        )
                        nc.tensor.matmul(
                            pu[:, 3, i * 64:(i + 1) * 64], lhsT,
                            wmat_sb[:, 8, :],
                            start=True, stop=True,
                        )
                    u_sb = upool.tile([128, K, UT, 64], BF16, tag="u_sb")
                    nc.scalar.copy(
                        u_sb[:, 0:8, 0:n, :],
                        pu[:, 0:n, :].rearrange("p i (k o) -> p k i o", o=64),
                    )
                    nc.vector.tensor_copy(
                        u_sb[:, 8, 0:n, :],
                        pu[:, 3, 0:n * 64].rearrange("p (i o) -> p i o", o=64),
                    )
                    # one DMA per round: all taps, rows 1 + h*W + (t0..t0+n)
                    nc.sync.dma_start(
                        out=_ap(utab[:],
                                [[W * O, 128], [ROWS * O, K], [1, n * O]],
                                (0 * ROWS + 1) * O + t0 * O),
                        in_=u_sb[:, :, 0:n, :],
                    )

        # ---- P2b: build vertical-pair table via DRAM->DRAM copies ----
        for k in range(K):
            src0 = k * ROWS * O + O          # u(0, 0)
            dst = k * NRG * 128
            # slot0: rows R(y, x) <- u(y, x), y in [0,127]
            nc.sync.dma_start(
                out=_ap(utabp[:], [[WP * 128, H], [128, W], [1, 64]],
                        dst + (1 * WP + 1) * 128),
                in_=_ap(utab[:], [[W * O, H], [O, W], [1, O]], src0),
            )
            # slot1: rows R(y', x) <- u(y'+1, x), y' in [-1,126]
            nc.scalar.dma_start(
                out=_ap(utabp[:], [[WP * 128, H], [128, W], [1, 64]],
                        dst + (0 * WP + 1) * 128 + 64),
                in_=_ap(utab[:], [[W * O, H], [O, W], [1, O]], src0),
            )

        # ---- P3: beta weights + gather indices ----
        with tc.tile_pool(name="scr", bufs=1) as scr:
            def S(tag, dt=FP32):
                return scr.tile([H, K, W], dt, tag=tag, name=tag)

            msig = S("msig")
            py = S("py")
            px = S("px")
            tmp = S("tmp")
            fi = S("fi", I16)
            fyf = S("fyf")
            fxf = S("fxf")
            wy = S("wy")
            wx = S("wx")
            ga = S("ga")
            gb = S("gb")
            ay0 = S("ay0")
            ay1 = S("ay1")
            ax0 = S("ax0")
            ax1 = S("ax1")
            cc = S("cc")
            idxi = scr.tile([H, K, W], I16, tag="idxi")

            ts = nc.vector.tensor_scalar
            tt = nc.vector.tensor_tensor
            stt = nc.vector.scalar_tensor_tensor

            nc.scalar.activation(msig[:], off_sb[:, 18:27, :],
                                 mybir.ActivationFunctionType.Sigmoid)
            # clip offsets, add base grid
            offv = off_sb[:, 0:18, :].rearrange("p (a b) w -> p a b w", b=2)
            ts(py[:], offv[:, 0:9, 0, :], -MAX_OFF, MAX_OFF, AL.max, AL.min)
            ts(px[:], offv[:, 0:9, 1, :], -MAX_OFF, MAX_OFF, AL.max, AL.min)
            tt(py[:], py[:], basey_sb[:], AL.add)
            tt(px[:], px[:], basex_sb[:], AL.add)

            def floor_frac(p, ff, w_frac):
                # HW DVE float->int converts round-to-nearest:
                # rint(p+63.5)-64 == floor(p) for p >= -63 (integer-p ties
                # resolve to floor or floor-1, both bilinear-equivalent).
                ts(tmp[:], p[:], 63.5, None, AL.add)
                nc.vector.tensor_copy(fi[:], tmp[:])      # fp32 -> int16 trunc
                nc.vector.tensor_copy(ff[:], fi[:])       # back to fp32
                ts(ff[:], ff[:], -64.0, None, AL.add)
                tt(w_frac[:], p[:], ff[:], AL.subtract)

            floor_frac(py, fyf, wy)
            floor_frac(px, fxf, wx)

            def edge_weights(ff, hi0, a0, a1, w_frac):
                # a0 = (1-w)*[lo<=f<=hi], a1 = w*[lo-1<=f<=hi-1], lo=0
                ts(ga[:], ff[:], 0.0, None, AL.is_ge)
                ts(gb[:], ff[:], hi0, None, AL.is_le)
                tt(ga[:], ga[:], gb[:], AL.mult)                  # valid0
                ts(a0[:], w_frac[:], -1.0, 1.0, AL.mult, AL.add)  # 1-w
                tt(a0[:], a0[:], ga[:], AL.mult)
                ts(ga[:], ff[:], -1.0, None, AL.is_ge)
                ts(gb[:], ff[:], hi0 - 1.0, None, AL.is_le)
                tt(ga[:], ga[:], gb[:], AL.mult)                  # valid1
                tt(a1[:], w_frac[:], ga[:], AL.mult)

            edge_weights(fyf, 127.0, ay0, ay1, wy)
            edge_weights(fxf, float(W - 1), ax0, ax1, wx)

            tt(ay0[:], msig[:], ay0[:], AL.mult)   # m*(1-wy)*vy0
            tt(ay1[:], msig[:], ay1[:], AL.mult)

            # corner products -> bf16 pair-duplicated bp[h, k, jx*2+jy, w, 2]
            for jy, ayv in ((0, ay0), (1, ay1)):
                for jx, axv in ((0, ax0), (1, ax1)):
                    tt(cc[:], ayv[:], axv[:], AL.mult)
                    for d in range(2):
                        nc.vector.tensor_copy(
                            bp[:, :, jx * 2 + jy, :, d], cc[:])

            # index: (y0c+1)*WP + x0c + 1, clamps to [-1, 127]
            ts(ga[:], fyf[:], 127.0, -1.0, AL.min, AL.max)
            ts(gb[:], fxf[:], 127.0, -1.0, AL.min, AL.max)
            ts(gb[:], gb[:], float(WP + 1), None, AL.add)  # x0c + 131
            stt(wy[:], ga[:], float(WP), gb[:], AL.mult, AL.add)
            nc.vector.tensor_copy(idxi[:], wy[:])

            # stage indices to DRAM in (h, k, w) order
            nc.sync.dma_start(
                out=_ap(idxd[:], [[K * W, 128], [1, K * W]], 0),
                in_=idxi[:],
            )

        # ---- P4: gather + weighted combine ----
        with (
            tc.tile_pool(name="bpl", bufs=1) as bpool,
            tc.tile_pool(name="apl", bufs=2) as apool,
            tc.tile_pool(name="gp", bufs=2) as gp,
            tc.tile_pool(name="tp", bufs=2) as tp,
            tc.tile_pool(name="yp", bufs=1, space="PSUM") as yp,
        ):
            # Re-load indices into the gather's wrapped layout: index i lives
            # at [i%16, i//16] with i = w*128 + h, i.e. [h%16, w*8 + h//16].
            # DMA brings (r; j=h//16, w) with contiguous w; a strided DVE copy
            # interleaves to f = w*8 + j. Replicated into all 8 groups of 16
            # partitions for the 8 gpsimd cores.
            bplanes = {}
            for k in range(K):
                bt = bpool.tile([128, 8 * W], I16, tag=f"b{k}", name=f"b{k}")
                at = apool.tile([128, 8, W], I16, tag="at", name="at")
                for grp in range(8):
                    nc.sync.dma_start(
                        out=at[16 * grp:16 * (grp + 1), :, :],
                        in_=_ap(idxd[:],
                                [[K * W, 16], [16 * K * W, 8], [1, W]],
                                k * W),
                    )
                slb = bt[:]
                pstr = slb.ap[0][0]
                nc.vector.tensor_copy(
                    _ap(slb, [[pstr, 128], [1, 8], [8, W]], slb.offset),
                    at[:],
                )
                bplanes[k] = bt

            NJ = max(1, CW // 4)  # accumulation sub-blocks per chunk
            JW = CW // NJ         # w columns per sub-block
            gq = 0
            for c0 in range(0, W, CW):
                ypsums = [yp.tile([128, JW, 2, O], FP32, tag=f"yps{j}",
                                  name=f"yps{j}") for j in range(NJ)]
                n18 = 0
                for k in range(K):
                    g = gp.tile([128, CW, 256], BF16, tag="g", name="g")
                    nc.gpsimd.dma_gather(
                        g[:],
                        _ap(utabp[:], [[128, NRG], [1, 256]],
                            k * NRG * 128),
                        bplanes[k][:, c0 * 8:(c0 + CW) * 8],
                        NI, NI,
                        256,
                        elem_step=128,
                        single_packet=False,
                    )
                    gq += 1
                    tv = tp.tile([128, CW, 4, 64], BF16, tag="tv", name="tv")
                    gv = g[:].rearrange("p c (j o) -> p c j o", j=4)
                    for j in range(4):
                        bview = (bp[:, k, j, c0:c0 + CW, :]
                                 .unsqueeze(2)
                                 .broadcast_to((H, CW, 32, 2)))
                        nc.vector.tensor_tensor(
                            tv[:, :, j, :].rearrange(
                                "p c (r d) -> p c r d", d=2),
                            gv[:, :, j, :].rearrange(
                                "p c (r d) -> p c r d", d=2),
                            bview, AL.mult,
                        )
                    for half in range(2):
                        for j in range(NJ):
                            nc.tensor.matmul(
                                ypsums[j][:],
                                ident_sb[:],
                                tv[:, j * JW:(j + 1) * JW,
                                   2 * half:2 * half + 2, :],
                                start=(n18 == 0), stop=(n18 == 17),
                            )
                        n18 += 1
                for j in range(NJ):
                    dst = ysb[:, :, c0 + j * JW: c0 + (j + 1) * JW]
                    nc.scalar.copy(
                        dst,
                        ypsums[j][:, :, 0, :].rearrange("p w o -> p o w"))
                    nc.vector.tensor_tensor(
                        dst,
                        ypsums[j][:, :, 1, :].rearrange("p w o -> p o w"),
                        dst, AL.add)

        # ---- P5: output ----
        nc.sync.dma_start(
            out=_ap(ydram[:], [[W, 128], [H * W, O], [1, W]], 0),
            in_=ysb[:],
        )

    nc.compile()
    return nc


def host_inputs(xb, offset_w, offset_b, mod_w, mod_b, weight, W=128):
    """Per-core input map for one batch element xb (C,H,W)."""
    wconv = np.zeros((C + 1, K, 27), np.float32)
    for s in range(K):
        dy, dx = s // KS, s % KS
        # out(h,t,j) = sum_c w[j,c,dy,dx] * xp[c, h+dy, t+dx]
        wconv[0:C, s, 0:18] = offset_w[:, :, dy, dx].T
        wconv[0:C, s, 18:27] = mod_w[:, :, dy, dx].T
    wconv[C, 0, 0:18] = offset_b
    wconv[C, 0, 18:27] = mod_b

    wmat = np.zeros((C, K, O), np.float32)
    for k in range(K):
        ky, kx = k // KS, k % KS
        wmat[:, k, :] = 2.0 * weight[:, :, ky, kx].T  # (c,o); x2 modulator fold

    hh = np.arange(H, dtype=np.float32)
    ww = np.arange(W, dtype=np.float32)
    ky = np.repeat(np.arange(KS), KS).astype(np.float32)
    kx = np.tile(np.arange(KS), KS).astype(np.float32)
    basey = np.broadcast_to(
        ky[None, :, None] + hh[:, None, None] - 1.0, (H, K, W))
    basex = np.broadcast_to(
        kx[None, :, None] + ww[None, None, :] - 1.0, (H, K, W))

    return {
        "xin": np.ascontiguousarray(xb, np.float32),
        "wconv": wconv.astype(ml_dtypes.bfloat16),
        "wmat": wmat.astype(ml_dtypes.bfloat16),
        "ident": np.eye(128, dtype=ml_dtypes.bfloat16),
        "basey": np.ascontiguousarray(basey, np.float32),
        "basex": np.ascontiguousarray(basex, np.float32),
    }


_prog_cache = {}


def _get_program(W=128, chunk_w=32):
    # 2048-descriptor gathers: two gathers' descriptor sets fit the SWDGE
    # ring, letting desc-gen of gather n+1 overlap the SDMA drain of n.
    chunk_w = min(chunk_w, 16)
    key = (W, chunk_w)
    if key not in _prog_cache:
        _prog_cache[key] = build_program(W=W, chunk_w=chunk_w)
    return _prog_cache[key]


def kernel(x, offset_w, offset_b, mod_w, mod_b, weight, trace=False):
    x = np.asarray(x, np.float32)
    B = x.shape[0]
    Wd = x.shape[3]
    nc = _get_program(W=Wd, chunk_w=min(32, Wd))
    in_maps = [
        host_inputs(x[b], np.asarray(offset_w, np.float32),
                    np.asarray(offset_b, np.float32),
                    np.asarray(mod_w, np.float32),
                    np.asarray(mod_b, np.float32),
                    np.asarray(weight, np.float32), W=Wd)
        for b in range(B)
    ]
    res = run_bass_kernel_spmd(nc, in_maps, list(range(B)), trace=trace)
    y = np.stack([res.results[b]["y"] for b in range(B)]).astype(np.float32)
    if trace:
        kernel.last_result = res
    return y
